# revision 43
# baseline (speedup 1.0000x reference)
"""ContraNorm kernel for 8x Trainium2 NeuronCores (Bass/Tile).

Computes, for x [8192, 512] fp32 (gamma/beta [512]):
    xn  = x / max(||x||_row, eps)
    sim = xn @ xn.T
    sim = softmax(sim, axis=1) + softmax(sim, axis=0)
    y   = x - 0.1 * (sim @ x)
    out = LayerNorm(y) * gamma + beta          (eps = 1e-6)

Key structure:
  * sim entries are cosine similarities in [-1, 1]; exp() never
    overflows so softmax needs no max-subtraction.  E = exp(sim) is
    symmetric, so column sums equal row sums:
        sim' = E * (1/r_i + 1/r_j)   (elementwise),  r = row sums of E
  * Row-shard across 8 cores.  Core q owns rows [q*1024, (q+1)*1024)
    and computes E^T tiles T[j, i] (j = all 8192 on partitions, i = its
    1024 rows on the free dim).  The full E^T strip (16.8 MB bf16,
    128 KB/partition) stays RESIDENT in SBUF between the phases - no
    DRAM round trip.
  * Both big matmuls run in fp8e4 with MatmulPerfMode.DoubleRow (2
    contraction subtiles per instruction).  A numpy model of the
    quantization points gives ~1e-4 max rel err vs the fp32 reference.
  * Own-row full sums r_i are computed locally with a ones-vector
    matmul over the E^T strip (no ReduceScatter needed).
  * invn for all rows comes from an AllGather of locally computed
    own-row inverse norms (4KB per core).
  * The AllReduce of partial row sums (for 1/r_j) is split in halves:
    the first is issued mid-phase-A, the second overlaps the first half
    of phase C.  Ops that consume collective results are kept off the
    hot ACT stream (the tile scheduler treats collectives as instant
    and will otherwise splice them into it).
"""

import sys

if "/opt/trn_rl_repo" not in sys.path:
    sys.path.insert(0, "/opt/trn_rl_repo")

import ml_dtypes
import numpy as np

import concourse.bass as bass
import concourse.tile as tile
from concourse import bacc, mybir
from concourse.bass_utils import run_bass_kernel_spmd

N = 8192
D = 512
N_CORES = 8
B = N // N_CORES          # 1024 rows per core
P = 128
JC = N // P               # 64 j-chunks
IT = B // 512             # 2 i-halves of the per-core block
ISUB = B // P             # 8 output row-subtiles
KO = D // P               # 4 contraction chunks
SCALE = 0.1
LN_EPS = 1e-6
XN_S = 16.0               # fp8 scale for xn moving operand
P_S = 4096.0              # fp8 scale for the p stationary operand

F32 = mybir.dt.float32
BF16 = mybir.dt.bfloat16
FP8 = mybir.dt.float8e4
AF = mybir.ActivationFunctionType
DR = mybir.MatmulPerfMode.DoubleRow


def build_kernel(reps=1):
    nc = bacc.Bacc("TRN2", target_bir_lowering=False, debug=False,
                   num_devices=N_CORES)

    # ---- I/O ----
    xT8 = nc.dram_tensor("xT8", [D, N], FP8, kind="ExternalInput")      # x.T fp8
    xb8 = nc.dram_tensor("xb8", [N, D], FP8, kind="ExternalInput")      # x fp8
    xTq = nc.dram_tensor("xTq", [D, B], BF16, kind="ExternalInput")     # x.T own cols
    xq = nc.dram_tensor("xq", [B, D], BF16, kind="ExternalInput")       # own rows bf16
    gamma = nc.dram_tensor("gamma", [D], F32, kind="ExternalInput")
    beta = nc.dram_tensor("beta", [D], F32, kind="ExternalInput")
    out = nc.dram_tensor("out", [B, D], F32, kind="ExternalOutput")

    xT8_v = xT8.ap().rearrange("(ko p) j -> p ko j", p=P)     # [128, 4, 8192]
    xTq_v = xTq.ap().rearrange("(ko p) i -> p ko i", p=P)     # [128, 4, 1024]
    xb8_v = xb8.ap().rearrange("(c p) d -> p c d", p=P)       # [128, 64, 512]
    xq_v = xq.ap().rearrange("(c p) d -> p c d", p=P)         # [128, 8, 512]

    with tile.TileContext(nc) as tc:
        for rep in range(reps):
            _body(nc, tc, xT8_v, xTq_v, xb8_v, xq_v, gamma, beta, out,
                  sfx=f"r{rep}")
    nc.compile()
    return nc


def _body(nc, tc, xT8_v, xTq_v, xb8_v, xq_v, gamma, beta, out, sfx=""):
    from contextlib import ExitStack
    ctx = ExitStack()
    rg = [list(range(N_CORES))]
    HJ = JC // 2              # 32 chunks per AllReduce half
    with ctx:
        persist = ctx.enter_context(tc.tile_pool(name=f"persist{sfx}", bufs=1))
        small = ctx.enter_context(tc.tile_pool(name=f"small{sfx}", bufs=2))
        stream = ctx.enter_context(tc.tile_pool(name=f"stream{sfx}", bufs=3))
        dram = ctx.enter_context(tc.tile_pool(name=f"dram{sfx}", bufs=1, space="DRAM"))

        # ---------- load persistent operands (xq first: it gates the norms) ----------
        xq_sb = persist.tile([P, ISUB, D], BF16)      # own rows bf16 (1 MB)
        nc.sync.dma_start(xq_sb[:], xq_v[:])

        gamma_b = persist.tile([P, D], F32)
        nc.sync.dma_start(gamma_b[:], bass.AP(tensor=gamma, offset=0,
                                              ap=[[0, P], [1, D]]))
        beta_b = persist.tile([P, D], F32)
        nc.sync.dma_start(beta_b[:], bass.AP(tensor=beta, offset=0,
                                             ap=[[0, P], [1, D]]))

        ones_sb = persist.tile([P, 1], BF16)
        nc.vector.memset(ones_sb[:], 1.0)

        # ---------- own-row inverse norms (local) ----------
        sq_scr = small.tile([P, D], BF16, tag="sqscr")
        ss_q = persist.tile([P, ISUB], F32)
        for t in range(ISUB):
            nc.scalar.activation(out=sq_scr[:], in_=xq_sb[:, t, :], func=AF.Square,
                                 accum_out=ss_q[:, t:t + 1])
        # invn/16 = exp(-0.5*ln(ss) - ln16), invn*16 = exp(-0.5*ln(ss) + ln16)
        # (avoids Sqrt: keeps a single ACT table for the whole kernel)
        ln16_n = small.tile([P, 1], F32, tag="ln16n")
        nc.vector.memset(ln16_n[:], -float(np.log(XN_S)))
        ln16_p = small.tile([P, 1], F32, tag="ln16p")
        nc.vector.memset(ln16_p[:], float(np.log(XN_S)))
        ln_q = small.tile([P, ISUB], F32, tag="lnq")
        nc.scalar.activation(out=ln_q[:], in_=ss_q[:], func=AF.Ln)
        invn_qs = small.tile([P, ISUB], F32, tag="invnqs")
        nc.scalar.activation(out=invn_qs[:], in_=ln_q[:], func=AF.Exp,
                             scale=-0.5, bias=ln16_n[:])
        invn_q16 = small.tile([P, ISUB], BF16, tag="invnq16")
        nc.scalar.activation(out=invn_q16[:], in_=ln_q[:], func=AF.Exp,
                             scale=-0.5, bias=ln16_p[:])

        # broadcast layout [P, B] of own invn*16 (via DRAM round trip)
        d_invnq = dram.tile([B], F32)         # invn/16 (AllGather payload)
        nc.gpsimd.dma_start(d_invnq.rearrange("(c p) -> p c", p=P), invn_qs[:])
        d_invnq16 = dram.tile([B], BF16)
        nc.gpsimd.dma_start(d_invnq16.rearrange("(c p) -> p c", p=P), invn_q16[:])
        # normalized own columns, fp8: xn8[d, i] = xTq[d, i] * invn[i] * 16
        # (bf16 multiply in DVE fast mode, fp8 convert on ACT; scratch in a
        # short-lived pool so the SBUF is returned before phase C's pools;
        # xn8 itself lives in a pool scoped to phase A)
        xn8_ctx = ExitStack()
        xn8p = xn8_ctx.enter_context(tc.tile_pool(name=f"xn8p{sfx}", bufs=1))
        xn8 = xn8p.tile([P, KO, B], FP8)
        with tc.tile_pool(name=f"prep{sfx}", bufs=1) as prep:
            invnq16_b = prep.tile([P, B], BF16)
            nc.scalar.dma_start(invnq16_b[:],
                                bass.AP(tensor=d_invnq16.tensor,
                                        offset=d_invnq16.offset,
                                        ap=[[0, P], [1, B]]))
            xTq_sb = prep.tile([P, KO, B], BF16)
            nc.sync.dma_start(xTq_sb[:], xTq_v[:])
            xn8a = prep.tile([P, KO, B], BF16)
            nc.vector.tensor_tensor(
                out=xn8a[:], in0=xTq_sb[:],
                in1=invnq16_b[:, None, :].to_broadcast((P, KO, B)),
                op=mybir.AluOpType.mult)
            nc.scalar.copy(out=xn8[:], in_=xn8a[:])

        # AllGather own invn/16 -> exp scale for all 8192 rows, [P, JC] layout
        c_n_all = dram.tile([N], F32)
        nc.gpsimd.collective_compute("AllGather", mybir.AluOpType.bypass,
                                     replica_groups=rg,
                                     ins=[d_invnq.opt()], outs=[c_n_all.opt()])
        invn_all_s = persist.tile([P, JC], F32)
        nc.scalar.dma_start(invn_all_s[:], c_n_all.rearrange("(c p) -> p c", p=P))

        # ---------- phase A: E^T tiles (SBUF-resident) + row sums ----------
        e_res = persist.tile([P, JC, B], BF16)        # 16.8 MB: 128 KB/partition
        sacc_lo = persist.tile([P, HJ], F32)          # partial row sums, chunks 0-31
        sacc_hi = persist.tile([P, HJ], F32)          # chunks 32-63
        c_lo = dram.tile([N // 2], F32)
        c_hi = dram.tile([N // 2], F32)
        c_lo_o = dram.tile([N // 2], F32)
        c_hi_o = dram.tile([N // 2], F32)

        with tc.tile_pool(name=f"psum_a{sfx}", bufs=3, space="PSUM") as psum_a, \
             tc.tile_pool(name=f"psum_rq{sfx}", bufs=1, space="PSUM") as psum_rq:
            rq_ps = psum_rq.tile([1, B], F32, tag="rqps", name=f"rqps{sfx}")
            for jq in range(JC // 4):
                xt8 = stream.tile([P, KO, 512], FP8, tag="xt8")
                nc.sync.dma_start(xt8[:], xT8_v[:, :, jq * 512:(jq + 1) * 512])
                for jj in range(4):
                    jc = jq * 4 + jj
                    sacc = sacc_lo if jc < HJ else sacc_hi
                    sc = jc if jc < HJ else jc - HJ
                    pt = psum_a.tile([P, B], F32, tag="ph_a")
                    for k2 in range(KO // 2):
                        for it in range(IT):
                            nc.tensor.matmul(
                                pt[:, it * 512:(it + 1) * 512],
                                xt8[:, 2 * k2:2 * k2 + 2, jj * P:(jj + 1) * P],
                                xn8[:, 2 * k2:2 * k2 + 2, it * 512:(it + 1) * 512],
                                start=(k2 == 0), stop=(k2 == KO // 2 - 1),
                                perf_mode=DR)
                    nc.scalar.activation(out=e_res[:, jc, :], in_=pt[:],
                                         func=AF.Exp,
                                         scale=invn_all_s[:, jc:jc + 1],
                                         accum_out=sacc[:, sc:sc + 1])
                # own-row sums: rq_ps[0, i] += sum_j E^T[j, i] (full, local).
                # Batched per group (PE otherwise flips DoubleRow<->regular
                # mode every chunk) and deferred by one group so every exp
                # dependency is old by the time PE reaches these
                for g in ([jq - 1] if jq > 0 else []) + ([jq] if jq == JC // 4 - 1 else []):
                    for jj in range(4):
                        jc = g * 4 + jj
                        for h in range(2):
                            nc.tensor.matmul(rq_ps[:, h * 512:(h + 1) * 512],
                                             ones_sb[:],
                                             e_res[:, jc, h * 512:(h + 1) * 512],
                                             start=(jc == 0), stop=(jc == JC - 1))
                if jq == (JC // 8) - 1:
                    # first half of partial row sums complete -> AllReduce now
                    nc.gpsimd.dma_start(c_lo.rearrange("(c p) -> p c", p=P),
                                        sacc_lo[:])
                    nc.gpsimd.collective_compute(
                        "AllReduce", mybir.AluOpType.add, replica_groups=rg,
                        ins=[c_lo.opt()], outs=[c_lo_o.opt()])

            # 4096/r for chunks 0-31 (DVE: idle mid-phase-A, so the AR_lo
            # wait cannot stall anything that matters)
            r_lo = small.tile([P, HJ], F32, tag="rlo")
            nc.gpsimd.dma_start(r_lo[:], c_lo_o.rearrange("(c p) -> p c", p=P))
            r_lo_s = small.tile([P, HJ], F32, tag="rlos")
            nc.vector.tensor_scalar(out=r_lo_s[:], in0=r_lo[:],
                                    scalar1=1.0 / P_S, scalar2=None,
                                    op0=mybir.AluOpType.mult)
            invr_lo = persist.tile([P, HJ], F32)
            nc.vector.reciprocal(out=invr_lo[:], in_=r_lo_s[:])

            # broadcast layout (for 4096/r_i): from the local ones-matmul sums
            invrq_f = small.tile([1, B], F32, tag="invrqf")
            nc.vector.reciprocal(out=invrq_f[:], in_=rq_ps[:])
            invrq_1 = small.tile([1, B], BF16, tag="invrq1")
            nc.vector.tensor_scalar(out=invrq_1[:], in0=invrq_f[:],
                                    scalar1=P_S, scalar2=None,
                                    op0=mybir.AluOpType.mult)
            # partition-broadcast via DRAM round trip on SP.  Pool must stay
            # a pure collective lane (anything queued there can land behind
            # an AllReduce engine-hold).
            d_invrq = dram.tile([B], BF16)
            nc.sync.dma_start(d_invrq.rearrange("(c b) -> c b", c=1), invrq_1[:])
            invrq_b = persist.tile([P, B], BF16)
            nc.sync.dma_start(invrq_b[:], bass.AP(tensor=d_invrq.tensor,
                                                  offset=d_invrq.offset,
                                                  ap=[[0, P], [1, B]]))

            nc.gpsimd.dma_start(c_hi.rearrange("(c p) -> p c", p=P), sacc_hi[:])
            nc.gpsimd.collective_compute("AllReduce", mybir.AluOpType.add,
                                         replica_groups=rg,
                                         ins=[c_hi.opt()], outs=[c_hi_o.opt()])

        xn8_ctx.close()    # free xn8 (phase-A only) before phase C's pools

        # ---------- phase C: x_neg = P^T.T @ x ----------
        pwork = ctx.enter_context(tc.tile_pool(name=f"pwork{sfx}", bufs=2))
        p8p = ctx.enter_context(tc.tile_pool(name=f"p8p{sfx}", bufs=2))
        lnw = ctx.enter_context(tc.tile_pool(name=f"lnw{sfx}", bufs=2))
        invr_hi = persist.tile([P, HJ], F32)
        psum_c = ctx.enter_context(
            tc.tile_pool(name=f"psum_c{sfx}", bufs=1, space="PSUM"))
        acc = [psum_c.tile([P, D], F32, tag=f"acc{i}", name=f"acc{i}{sfx}")
               for i in range(ISUB)]
        for jq in range(JC // 4):
            if jq == JC // 8:
                # 4096/r for chunks 32-63; extra DRAM hops delay this
                # chain's scheduling-pass readiness past m_t(group 0)
                d_rh2 = dram.tile([N // 2], F32)
                nc.gpsimd.dma_start(d_rh2.rearrange("(a b) -> a b", a=1),
                                    c_hi_o.rearrange("(a b) -> a b", a=1))
                d_rh3 = dram.tile([N // 2], F32)
                nc.gpsimd.dma_start(d_rh3.rearrange("(a b) -> a b", a=1),
                                    d_rh2.rearrange("(a b) -> a b", a=1))
                r_hi = small.tile([P, HJ], F32, tag="rhi")
                nc.gpsimd.dma_start(r_hi[:],
                                    d_rh3.rearrange("(c p) -> p c", p=P))
                r_hi_s = small.tile([P, HJ], F32, tag="rhis")
                nc.vector.tensor_scalar(out=r_hi_s[:], in0=r_hi[:],
                                        scalar1=1.0 / P_S, scalar2=None,
                                        op0=mybir.AluOpType.mult)
                nc.vector.reciprocal(out=invr_hi[:], in_=r_hi_s[:])
            xb4 = stream.tile([P, 4, D], FP8, tag="xb4_c")
            nc.sync.dma_start(xb4[:], xb8_v[:, jq * 4:(jq + 1) * 4, :])
            m_t = pwork.tile([P, 4, B], BF16, tag="mt")
            for jj in range(4):
                jc = jq * 4 + jj
                invr = invr_lo if jc < HJ else invr_hi
                sc = jc if jc < HJ else jc - HJ
                nc.vector.tensor_scalar(out=m_t[:, jj, :], in0=invrq_b[:],
                                        scalar1=invr[:, sc:sc + 1],
                                        scalar2=None, op0=mybir.AluOpType.add)
            # p = E * m, in place over m (E^T read straight from SBUF)
            nc.vector.tensor_tensor(out=m_t[:], in0=m_t[:],
                                    in1=e_res[:, jq * 4:(jq + 1) * 4, :],
                                    op=mybir.AluOpType.mult)
            p8 = p8p.tile([P, 4, B], FP8, tag="p8")
            nc.scalar.copy(out=p8[:], in_=m_t[:])
            for j2 in range(2):
                last = (jq == JC // 4 - 1) and (j2 == 1)
                for i in range(ISUB):
                    nc.tensor.matmul(acc[i][:],
                                     p8[:, 2 * j2:2 * j2 + 2, i * P:(i + 1) * P],
                                     xb4[:, 2 * j2:2 * j2 + 2, :],
                                     start=(jq == 0 and j2 == 0), stop=last,
                                     perf_mode=DR)

        # ---------- tail: y = xq - 0.1*x_neg/4096 ; LayerNorm ----------
        # batched Ln/Exp (one each): per-subtile Ln/Exp alternation makes the
        # compiler ping-pong activation-table loads, serializing the tail
        eps_t = small.tile([P, 1], F32, tag="eps")
        nc.vector.memset(eps_t[:], LN_EPS)
        out_v = out.ap().rearrange("(c p) d -> p c d", p=P)
        y_all = persist.tile([P, ISUB, D], BF16)
        mv_all = persist.tile([P, ISUB, 2], F32)
        for i in range(ISUB):
            nc.vector.tensor_scalar(out=y_all[:, i, :], in0=acc[i][:],
                                    scalar1=-SCALE / P_S,
                                    scalar2=None, op0=mybir.AluOpType.mult)
            nc.vector.tensor_tensor(out=y_all[:, i, :], in0=y_all[:, i, :],
                                    in1=xq_sb[:, i, :], op=mybir.AluOpType.add)
            stats = lnw.tile([P, 6], F32, tag="stats")
            nc.vector.bn_stats(out=stats[:], in_=y_all[:, i, :])
            nc.vector.bn_aggr(out=mv_all[:, i, :], in_=stats[:])
        lnv = small.tile([P, ISUB], F32, tag="lnv")
        nc.scalar.activation(out=lnv[:], in_=mv_all[:, :, 1], func=AF.Ln,
                             bias=eps_t[:])
        rstd_all = small.tile([P, ISUB], F32, tag="rstdall")
        nc.scalar.activation(out=rstd_all[:], in_=lnv[:], func=AF.Exp,
                             scale=-0.5)
        for i in range(ISUB):
            o_t = lnw.tile([P, D], F32, tag="o")
            nc.vector.tensor_scalar(out=o_t[:], in0=y_all[:, i, :],
                                    scalar1=mv_all[:, i, 0:1],
                                    scalar2=rstd_all[:, i:i + 1],
                                    op0=mybir.AluOpType.subtract,
                                    op1=mybir.AluOpType.mult)
            nc.vector.tensor_tensor(out=o_t[:], in0=o_t[:],
                                    in1=gamma_b[:], op=mybir.AluOpType.mult)
            nc.vector.tensor_tensor(out=o_t[:], in0=o_t[:],
                                    in1=beta_b[:], op=mybir.AluOpType.add)
            nc.sync.dma_start(out_v[:, i, :], o_t[:])


_CACHE = {}


def _get_nc():
    if "nc" not in _CACHE:
        _CACHE["nc"] = build_kernel()
    return _CACHE["nc"]


def make_in_maps(x, gamma, beta):
    x = np.asarray(x, dtype=np.float32)
    fp8 = ml_dtypes.float8_e4m3
    bf = ml_dtypes.bfloat16
    xT = np.ascontiguousarray(x.T)
    xT8 = xT.astype(fp8)
    xb8 = x.astype(fp8)
    xT_bf = xT.astype(bf)
    x_bf = x.astype(bf)
    gamma = np.asarray(gamma, dtype=np.float32)
    beta = np.asarray(beta, dtype=np.float32)
    in_maps = []
    for q in range(N_CORES):
        sl = slice(q * B, (q + 1) * B)
        in_maps.append({
            "xT8": xT8,
            "xb8": xb8,
            "xTq": np.ascontiguousarray(xT_bf[:, sl]),
            "xq": np.ascontiguousarray(x_bf[sl]),
            "gamma": gamma,
            "beta": beta,
        })
    return in_maps


def kernel(x, gamma, beta):
    nc = _get_nc()
    in_maps = make_in_maps(x, gamma, beta)
    res = run_bass_kernel_spmd(nc, in_maps, core_ids=list(range(N_CORES)))
    out = np.concatenate([res.results[q]["out"] for q in range(N_CORES)], axis=0)
    return out.astype(np.float32)


if __name__ == "__main__":
    rng = np.random.default_rng(0)
    x = rng.standard_normal((N, D), dtype=np.float32)
    gamma = np.ones(D, np.float32)
    beta = np.zeros(D, np.float32)
    o = kernel(x, gamma, beta)
    print("out", o.shape, o.dtype, float(np.abs(o).mean()))


# revision 44
# speedup vs baseline: 1.1532x; 1.1532x over previous
"""ContraNorm kernel for 8x Trainium2 NeuronCores (Bass/Tile).

Computes, for x [8192, 512] fp32 (gamma/beta [512]):
    xn  = x / max(||x||_row, eps)
    sim = xn @ xn.T
    sim = softmax(sim, axis=1) + softmax(sim, axis=0)
    y   = x - 0.1 * (sim @ x)
    out = LayerNorm(y) * gamma + beta          (eps = 1e-6)

Key structure:
  * sim entries are cosine similarities in [-1, 1]; exp() never
    overflows so softmax needs no max-subtraction.  E = exp(sim) is
    symmetric, so column sums equal row sums:
        sim' = E * (1/r_i + 1/r_j)   (elementwise),  r = row sums of E
  * Row-shard across 8 cores.  Core q owns rows [q*1024, (q+1)*1024)
    and computes E^T tiles T[j, i] (j = all 8192 on partitions, i = its
    1024 rows on the free dim).  The full E^T strip (16.8 MB bf16,
    128 KB/partition) stays RESIDENT in SBUF between the phases - no
    DRAM round trip.
  * Both big matmuls run in fp8e4 with MatmulPerfMode.DoubleRow (2
    contraction subtiles per instruction).  A numpy model of the
    quantization points gives ~1e-4 max rel err vs the fp32 reference.
  * Own-row full sums r_i are computed locally with a ones-vector
    matmul over the E^T strip (no ReduceScatter needed).
  * invn for all rows comes from an AllGather of locally computed
    own-row inverse norms (4KB per core).
  * The AllReduce of partial row sums (for 1/r_j) is split in halves:
    the first is issued mid-phase-A, the second overlaps the first half
    of phase C.  Ops that consume collective results are kept off the
    hot ACT stream (the tile scheduler treats collectives as instant
    and will otherwise splice them into it).
"""

import sys

if "/opt/trn_rl_repo" not in sys.path:
    sys.path.insert(0, "/opt/trn_rl_repo")

import ml_dtypes
import numpy as np

import concourse.bass as bass
import concourse.tile as tile
from concourse import bacc, mybir
from concourse.bass_utils import run_bass_kernel_spmd

N = 8192
D = 512
N_CORES = 8
B = N // N_CORES          # 1024 rows per core
P = 128
JC = N // P               # 64 j-chunks
IT = B // 512             # 2 i-halves of the per-core block
ISUB = B // P             # 8 output row-subtiles
KO = D // P               # 4 contraction chunks
SCALE = 0.1
LN_EPS = 1e-6
XN_S = 16.0               # fp8 scale for xn moving operand
P_S = 4096.0              # fp8 scale for the p stationary operand

F32 = mybir.dt.float32
BF16 = mybir.dt.bfloat16
FP8 = mybir.dt.float8e4
AF = mybir.ActivationFunctionType
DR = mybir.MatmulPerfMode.DoubleRow


def build_kernel(reps=1):
    nc = bacc.Bacc("TRN2", target_bir_lowering=False, debug=False,
                   num_devices=N_CORES)

    # ---- I/O ----
    xT8 = nc.dram_tensor("xT8", [D, N], FP8, kind="ExternalInput")      # x.T fp8
    xb8 = nc.dram_tensor("xb8", [N, D], FP8, kind="ExternalInput")      # x fp8
    xTq = nc.dram_tensor("xTq", [D, B], BF16, kind="ExternalInput")     # x.T own cols
    xq = nc.dram_tensor("xq", [B, D], BF16, kind="ExternalInput")       # own rows bf16
    gamma = nc.dram_tensor("gamma", [D], F32, kind="ExternalInput")
    beta = nc.dram_tensor("beta", [D], F32, kind="ExternalInput")
    out = nc.dram_tensor("out", [B, D], F32, kind="ExternalOutput")

    xT8_v = xT8.ap().rearrange("(ko p) j -> p ko j", p=P)     # [128, 4, 8192]
    xTq_v = xTq.ap().rearrange("(ko p) i -> p ko i", p=P)     # [128, 4, 1024]
    xb8_v = xb8.ap().rearrange("(c p) d -> p c d", p=P)       # [128, 64, 512]
    xq_v = xq.ap().rearrange("(c p) d -> p c d", p=P)         # [128, 8, 512]

    with tile.TileContext(nc) as tc:
        for rep in range(reps):
            _body(nc, tc, xT8_v, xTq_v, xb8_v, xq_v, gamma, beta, out,
                  sfx=f"r{rep}")
    nc.compile()
    return nc


def _body(nc, tc, xT8_v, xTq_v, xb8_v, xq_v, gamma, beta, out, sfx=""):
    from contextlib import ExitStack
    ctx = ExitStack()
    rg = [list(range(N_CORES))]
    HJ = JC // 2              # 32 chunks per AllReduce half
    with ctx:
        persist = ctx.enter_context(tc.tile_pool(name=f"persist{sfx}", bufs=1))
        small = ctx.enter_context(tc.tile_pool(name=f"small{sfx}", bufs=2))
        stream = ctx.enter_context(tc.tile_pool(name=f"stream{sfx}", bufs=3))
        dram = ctx.enter_context(tc.tile_pool(name=f"dram{sfx}", bufs=1, space="DRAM"))

        # ---------- load persistent operands (xq first: it gates the norms) ----------
        xq_sb = persist.tile([P, ISUB, D], BF16)      # own rows bf16 (1 MB)
        nc.sync.dma_start(xq_sb[:], xq_v[:])

        gamma_b = persist.tile([P, D], F32)
        nc.sync.dma_start(gamma_b[:], bass.AP(tensor=gamma, offset=0,
                                              ap=[[0, P], [1, D]]))
        beta_b = persist.tile([P, D], F32)
        nc.sync.dma_start(beta_b[:], bass.AP(tensor=beta, offset=0,
                                             ap=[[0, P], [1, D]]))

        ones_sb = persist.tile([P, 1], BF16)
        nc.vector.memset(ones_sb[:], 1.0)

        # ---------- own-row inverse norms (local) ----------
        sq_scr = small.tile([P, D], BF16, tag="sqscr")
        ss_q = persist.tile([P, ISUB], F32)
        for t in range(ISUB):
            nc.scalar.activation(out=sq_scr[:], in_=xq_sb[:, t, :], func=AF.Square,
                                 accum_out=ss_q[:, t:t + 1])
        # invn/16 = exp(-0.5*ln(ss) - ln16), invn*16 = exp(-0.5*ln(ss) + ln16)
        # (avoids Sqrt: keeps a single ACT table for the whole kernel)
        ln16_n = small.tile([P, 1], F32, tag="ln16n")
        nc.vector.memset(ln16_n[:], -float(np.log(XN_S)))
        ln16_p = small.tile([P, 1], F32, tag="ln16p")
        nc.vector.memset(ln16_p[:], float(np.log(XN_S)))
        ln_q = small.tile([P, ISUB], F32, tag="lnq")
        nc.scalar.activation(out=ln_q[:], in_=ss_q[:], func=AF.Ln)
        invn_qs = small.tile([P, ISUB], F32, tag="invnqs")
        nc.scalar.activation(out=invn_qs[:], in_=ln_q[:], func=AF.Exp,
                             scale=-0.5, bias=ln16_n[:])
        invn_q16 = small.tile([P, ISUB], BF16, tag="invnq16")
        nc.scalar.activation(out=invn_q16[:], in_=ln_q[:], func=AF.Exp,
                             scale=-0.5, bias=ln16_p[:])

        # broadcast layout [P, B] of own invn*16 (via DRAM round trip)
        d_invnq = dram.tile([B], F32)         # invn/16 (AllGather payload)
        nc.gpsimd.dma_start(d_invnq.rearrange("(c p) -> p c", p=P), invn_qs[:])
        d_invnq16 = dram.tile([B], BF16)
        nc.gpsimd.dma_start(d_invnq16.rearrange("(c p) -> p c", p=P), invn_q16[:])
        # normalized own columns, fp8: xn8[d, i] = xTq[d, i] * invn[i] * 16
        # (bf16 multiply in DVE fast mode, fp8 convert on ACT; scratch in a
        # short-lived pool so the SBUF is returned before phase C's pools;
        # xn8 itself lives in a pool scoped to phase A)
        xn8_ctx = ExitStack()
        xn8p = xn8_ctx.enter_context(tc.tile_pool(name=f"xn8p{sfx}", bufs=1))
        xn8 = xn8p.tile([P, KO, B], FP8)
        with tc.tile_pool(name=f"prep{sfx}", bufs=1) as prep:
            invnq16_b = prep.tile([P, B], BF16)
            nc.scalar.dma_start(invnq16_b[:],
                                bass.AP(tensor=d_invnq16.tensor,
                                        offset=d_invnq16.offset,
                                        ap=[[0, P], [1, B]]))
            xTq_sb = prep.tile([P, KO, B], BF16)
            nc.sync.dma_start(xTq_sb[:], xTq_v[:])
            xn8a = prep.tile([P, KO, B], BF16)
            nc.vector.tensor_tensor(
                out=xn8a[:], in0=xTq_sb[:],
                in1=invnq16_b[:, None, :].to_broadcast((P, KO, B)),
                op=mybir.AluOpType.mult)
            nc.scalar.copy(out=xn8[:], in_=xn8a[:])

        # AllGather own invn/16 -> exp scale for all 8192 rows, [P, JC] layout
        c_n_all = dram.tile([N], F32)
        nc.gpsimd.collective_compute("AllGather", mybir.AluOpType.bypass,
                                     replica_groups=rg,
                                     ins=[d_invnq.opt()], outs=[c_n_all.opt()])
        invn_all_s = persist.tile([P, JC], F32)
        nc.scalar.dma_start(invn_all_s[:], c_n_all.rearrange("(c p) -> p c", p=P))

        # ---------- phase A: E^T tiles (SBUF-resident) + row sums ----------
        e_res = persist.tile([P, JC, B], BF16)        # 16.8 MB: 128 KB/partition
        sacc_lo = persist.tile([P, HJ], F32)          # partial row sums, chunks 0-31
        sacc_hi = persist.tile([P, HJ], F32)          # chunks 32-63
        c_lo = dram.tile([N // 2], F32)
        c_hi = dram.tile([N // 2], F32)
        c_lo_o = dram.tile([N // 2], F32)
        c_hi_o = dram.tile([N // 2], F32)

        with tc.tile_pool(name=f"psum_a{sfx}", bufs=3, space="PSUM") as psum_a, \
             tc.tile_pool(name=f"psum_rq{sfx}", bufs=1, space="PSUM") as psum_rq:
            rq_ps = psum_rq.tile([1, B], F32, tag="rqps", name=f"rqps{sfx}")
            for jq in range(JC // 4):
                xt8 = stream.tile([P, KO, 512], FP8, tag="xt8")
                nc.sync.dma_start(xt8[:], xT8_v[:, :, jq * 512:(jq + 1) * 512])
                for jj in range(4):
                    jc = jq * 4 + jj
                    sacc = sacc_lo if jc < HJ else sacc_hi
                    sc = jc if jc < HJ else jc - HJ
                    pt = psum_a.tile([P, B], F32, tag="ph_a")
                    for k2 in range(KO // 2):
                        for it in range(IT):
                            nc.tensor.matmul(
                                pt[:, it * 512:(it + 1) * 512],
                                xt8[:, 2 * k2:2 * k2 + 2, jj * P:(jj + 1) * P],
                                xn8[:, 2 * k2:2 * k2 + 2, it * 512:(it + 1) * 512],
                                start=(k2 == 0), stop=(k2 == KO // 2 - 1),
                                perf_mode=DR)
                    nc.scalar.activation(out=e_res[:, jc, :], in_=pt[:],
                                         func=AF.Exp,
                                         scale=invn_all_s[:, jc:jc + 1],
                                         accum_out=sacc[:, sc:sc + 1])
                # own-row sums: rq_ps[0, i] += sum_j E^T[j, i] (full, local).
                # Batched per group: the PE otherwise flips between DoubleRow
                # and regular matmul mode every chunk (128 mode switches).
                # (Deferring by one group was tested and REGRESSED on HW.)
                for jj in range(4):
                    jc = jq * 4 + jj
                    for h in range(2):
                        nc.tensor.matmul(rq_ps[:, h * 512:(h + 1) * 512],
                                         ones_sb[:],
                                         e_res[:, jc, h * 512:(h + 1) * 512],
                                         start=(jc == 0), stop=(jc == JC - 1))
                if jq == (JC // 8) - 1:
                    # first half of partial row sums complete -> AllReduce now
                    nc.gpsimd.dma_start(c_lo.rearrange("(c p) -> p c", p=P),
                                        sacc_lo[:])
                    nc.gpsimd.collective_compute(
                        "AllReduce", mybir.AluOpType.add, replica_groups=rg,
                        ins=[c_lo.opt()], outs=[c_lo_o.opt()])

            # 4096/r for chunks 0-31 (DVE: idle mid-phase-A, so the AR_lo
            # wait cannot stall anything that matters)
            r_lo = small.tile([P, HJ], F32, tag="rlo")
            nc.gpsimd.dma_start(r_lo[:], c_lo_o.rearrange("(c p) -> p c", p=P))
            r_lo_s = small.tile([P, HJ], F32, tag="rlos")
            nc.vector.tensor_scalar(out=r_lo_s[:], in0=r_lo[:],
                                    scalar1=1.0 / P_S, scalar2=None,
                                    op0=mybir.AluOpType.mult)
            invr_lo = persist.tile([P, HJ], F32)
            nc.vector.reciprocal(out=invr_lo[:], in_=r_lo_s[:])

            # broadcast layout (for 4096/r_i): from the local ones-matmul sums
            invrq_f = small.tile([1, B], F32, tag="invrqf")
            nc.vector.reciprocal(out=invrq_f[:], in_=rq_ps[:])
            invrq_1 = small.tile([1, B], BF16, tag="invrq1")
            nc.vector.tensor_scalar(out=invrq_1[:], in0=invrq_f[:],
                                    scalar1=P_S, scalar2=None,
                                    op0=mybir.AluOpType.mult)
            # partition-broadcast via DRAM round trip on SP.  Pool must stay
            # a pure collective lane (anything queued there can land behind
            # an AllReduce engine-hold).
            d_invrq = dram.tile([B], BF16)
            nc.sync.dma_start(d_invrq.rearrange("(c b) -> c b", c=1), invrq_1[:])
            invrq_b = persist.tile([P, B], BF16)
            nc.sync.dma_start(invrq_b[:], bass.AP(tensor=d_invrq.tensor,
                                                  offset=d_invrq.offset,
                                                  ap=[[0, P], [1, B]]))

            nc.gpsimd.dma_start(c_hi.rearrange("(c p) -> p c", p=P), sacc_hi[:])
            nc.gpsimd.collective_compute("AllReduce", mybir.AluOpType.add,
                                         replica_groups=rg,
                                         ins=[c_hi.opt()], outs=[c_hi_o.opt()])

        xn8_ctx.close()    # free xn8 (phase-A only) before phase C's pools

        # ---------- phase C: x_neg = P^T.T @ x ----------
        pwork = ctx.enter_context(tc.tile_pool(name=f"pwork{sfx}", bufs=2))
        p8p = ctx.enter_context(tc.tile_pool(name=f"p8p{sfx}", bufs=2))
        lnw = ctx.enter_context(tc.tile_pool(name=f"lnw{sfx}", bufs=2))
        invr_hi = persist.tile([P, HJ], F32)
        psum_c = ctx.enter_context(
            tc.tile_pool(name=f"psum_c{sfx}", bufs=1, space="PSUM"))
        acc = [psum_c.tile([P, D], F32, tag=f"acc{i}", name=f"acc{i}{sfx}")
               for i in range(ISUB)]
        for jq in range(JC // 4):
            if jq == JC // 8:
                # 4096/r for chunks 32-63; extra DRAM hops delay this
                # chain's scheduling-pass readiness past m_t(group 0)
                d_rh2 = dram.tile([N // 2], F32)
                nc.gpsimd.dma_start(d_rh2.rearrange("(a b) -> a b", a=1),
                                    c_hi_o.rearrange("(a b) -> a b", a=1))
                d_rh3 = dram.tile([N // 2], F32)
                nc.gpsimd.dma_start(d_rh3.rearrange("(a b) -> a b", a=1),
                                    d_rh2.rearrange("(a b) -> a b", a=1))
                r_hi = small.tile([P, HJ], F32, tag="rhi")
                nc.gpsimd.dma_start(r_hi[:],
                                    d_rh3.rearrange("(c p) -> p c", p=P))
                r_hi_s = small.tile([P, HJ], F32, tag="rhis")
                nc.vector.tensor_scalar(out=r_hi_s[:], in0=r_hi[:],
                                        scalar1=1.0 / P_S, scalar2=None,
                                        op0=mybir.AluOpType.mult)
                nc.vector.reciprocal(out=invr_hi[:], in_=r_hi_s[:])
            xb4 = stream.tile([P, 4, D], FP8, tag="xb4_c")
            nc.sync.dma_start(xb4[:], xb8_v[:, jq * 4:(jq + 1) * 4, :])
            m_t = pwork.tile([P, 4, B], BF16, tag="mt")
            for jj in range(4):
                jc = jq * 4 + jj
                invr = invr_lo if jc < HJ else invr_hi
                sc = jc if jc < HJ else jc - HJ
                nc.vector.tensor_scalar(out=m_t[:, jj, :], in0=invrq_b[:],
                                        scalar1=invr[:, sc:sc + 1],
                                        scalar2=None, op0=mybir.AluOpType.add)
            # p = E * m, in place over m (E^T read straight from SBUF)
            nc.vector.tensor_tensor(out=m_t[:], in0=m_t[:],
                                    in1=e_res[:, jq * 4:(jq + 1) * 4, :],
                                    op=mybir.AluOpType.mult)
            p8 = p8p.tile([P, 4, B], FP8, tag="p8")
            nc.scalar.copy(out=p8[:], in_=m_t[:])
            for j2 in range(2):
                last = (jq == JC // 4 - 1) and (j2 == 1)
                for i in range(ISUB):
                    nc.tensor.matmul(acc[i][:],
                                     p8[:, 2 * j2:2 * j2 + 2, i * P:(i + 1) * P],
                                     xb4[:, 2 * j2:2 * j2 + 2, :],
                                     start=(jq == 0 and j2 == 0), stop=last,
                                     perf_mode=DR)

        # ---------- tail: y = xq - 0.1*x_neg/4096 ; LayerNorm ----------
        # batched Ln/Exp (one each): per-subtile Ln/Exp alternation makes the
        # compiler ping-pong activation-table loads, serializing the tail
        eps_t = small.tile([P, 1], F32, tag="eps")
        nc.vector.memset(eps_t[:], LN_EPS)
        out_v = out.ap().rearrange("(c p) d -> p c d", p=P)
        y_all = persist.tile([P, ISUB, D], BF16)
        mv_all = persist.tile([P, ISUB, 2], F32)
        for i in range(ISUB):
            nc.vector.tensor_scalar(out=y_all[:, i, :], in0=acc[i][:],
                                    scalar1=-SCALE / P_S,
                                    scalar2=None, op0=mybir.AluOpType.mult)
            nc.vector.tensor_tensor(out=y_all[:, i, :], in0=y_all[:, i, :],
                                    in1=xq_sb[:, i, :], op=mybir.AluOpType.add)
            stats = lnw.tile([P, 6], F32, tag="stats")
            nc.vector.bn_stats(out=stats[:], in_=y_all[:, i, :])
            nc.vector.bn_aggr(out=mv_all[:, i, :], in_=stats[:])
        lnv = small.tile([P, ISUB], F32, tag="lnv")
        nc.scalar.activation(out=lnv[:], in_=mv_all[:, :, 1], func=AF.Ln,
                             bias=eps_t[:])
        rstd_all = small.tile([P, ISUB], F32, tag="rstdall")
        nc.scalar.activation(out=rstd_all[:], in_=lnv[:], func=AF.Exp,
                             scale=-0.5)
        for i in range(ISUB):
            o_t = lnw.tile([P, D], F32, tag="o")
            nc.vector.tensor_scalar(out=o_t[:], in0=y_all[:, i, :],
                                    scalar1=mv_all[:, i, 0:1],
                                    scalar2=rstd_all[:, i:i + 1],
                                    op0=mybir.AluOpType.subtract,
                                    op1=mybir.AluOpType.mult)
            nc.vector.tensor_tensor(out=o_t[:], in0=o_t[:],
                                    in1=gamma_b[:], op=mybir.AluOpType.mult)
            nc.vector.tensor_tensor(out=o_t[:], in0=o_t[:],
                                    in1=beta_b[:], op=mybir.AluOpType.add)
            nc.sync.dma_start(out_v[:, i, :], o_t[:])


_CACHE = {}


def _get_nc():
    if "nc" not in _CACHE:
        _CACHE["nc"] = build_kernel()
    return _CACHE["nc"]


def make_in_maps(x, gamma, beta):
    x = np.asarray(x, dtype=np.float32)
    fp8 = ml_dtypes.float8_e4m3
    bf = ml_dtypes.bfloat16
    xT = np.ascontiguousarray(x.T)
    xT8 = xT.astype(fp8)
    xb8 = x.astype(fp8)
    xT_bf = xT.astype(bf)
    x_bf = x.astype(bf)
    gamma = np.asarray(gamma, dtype=np.float32)
    beta = np.asarray(beta, dtype=np.float32)
    in_maps = []
    for q in range(N_CORES):
        sl = slice(q * B, (q + 1) * B)
        in_maps.append({
            "xT8": xT8,
            "xb8": xb8,
            "xTq": np.ascontiguousarray(xT_bf[:, sl]),
            "xq": np.ascontiguousarray(x_bf[sl]),
            "gamma": gamma,
            "beta": beta,
        })
    return in_maps


def kernel(x, gamma, beta):
    nc = _get_nc()
    in_maps = make_in_maps(x, gamma, beta)
    res = run_bass_kernel_spmd(nc, in_maps, core_ids=list(range(N_CORES)))
    out = np.concatenate([res.results[q]["out"] for q in range(N_CORES)], axis=0)
    return out.astype(np.float32)


if __name__ == "__main__":
    rng = np.random.default_rng(0)
    x = rng.standard_normal((N, D), dtype=np.float32)
    gamma = np.ones(D, np.float32)
    beta = np.zeros(D, np.float32)
    o = kernel(x, gamma, beta)
    print("out", o.shape, o.dtype, float(np.abs(o).mean()))
